# revision 1
# baseline (speedup 1.0000x reference)
"""Causal multi-head attention for Trainium2, 8-core tensor-parallel over heads.

Problem: B=4, S=2048, D=1024, H=16 heads (dk=64), fp32.
    q = x @ w_q.T ; k = x @ w_k.T ; v = x @ w_v.T   (per-head split)
    out = softmax(causal(q k^T / 8)) v, concat heads, @ w_o.T + b_o

Sharding: core c owns heads (2c, 2c+1) = channels [128c, 128c+128).
Each core computes q/k/v projections for its two heads over the full batch,
runs causal attention, and produces a partial output projection
outT_c = (w_o[:, ch_c] a_c^T) of shape [1024, B*S]; the host sums the 8
partials, transposes, and adds b_o.

Per-core dataflow (all matmuls in fp32r = full PE rate, ~1e-4 rel err):
  - x is pre-transposed on host to xT [B, D, S] so the contraction dim D
    lands on SBUF partitions.
  - qT, kT [128ch, S] per batch via wT-stationary matmuls (heads stacked:
    h0 on partitions 0-63, h1 on 64-127).
  - v produced transposed like q/k then PE-transposed to natural [tok, ch]
    blocks, stored as [v_h | ones64] stationaries: the AV matmul
    out = [v|1]^T P then yields both the attention output (rows 0-63) and
    the softmax denominator replicated on rows 64-127 — no partition
    reduction needed anywhere.
  - scores are computed transposed (keys on partitions): sT = kT^T qT via
    row-packed matmuls (two heads concurrently in row groups 0-1/2-3).
  - softmax without max-subtraction (scores are ~N(0,1); exp is safe in
    fp32), causal mask only on diagonal 128x128 blocks via a precomputed
    additive -1e9 mask; fully-masked column ranges are memset to -1e9.
  - normalization: reciprocal of the replicated denominator rows then one
    elementwise multiply, written straight into the stacked aT layout that
    the output projection consumes.
"""

import numpy as np

import concourse.bass as bass
import concourse.tile as tile
from concourse import mybir
from concourse import bass_utils

f32 = mybir.dt.float32
f32r = mybir.dt.float32r
u32 = mybir.dt.uint32
AF = mybir.ActivationFunctionType

B, S, D, H = 4, 2048, 1024, 16
DK = D // H            # 64
NCORES = 8
PT = 128               # partition tile
CHUNK = 512            # query chunk (fp32 matmul max moving dim)
NEG = -1.0e9

_DMA_CLASSES = {"InstDMACopy", "InstTriggeredCopy", "InstDMATranspose", "InstDMAGatherAnt"}


def _split_multi_waits(nc):
    """This walrus build allows at most one sync-wait per TPB instruction;
    hoist extra waits onto single-wait NoOps on the same engine."""
    n = 0
    for f in nc.m.functions:
        for blk in f.blocks:
            new = []
            for inst in blk.instructions:
                si = inst.sync_info
                if si is not None and si.on_wait and len(si.on_wait) > 1:
                    ws = list(si.on_wait)
                    for w in ws[:-1]:
                        new.append(mybir.InstNoOp(
                            name=f"I-wfix-{n}", ins=[], outs=[], engine=inst.engine,
                            sync_info=mybir.SyncInfo(on_wait=[w], on_update=[])))
                        n += 1
                    inst.sync_info = mybir.SyncInfo(
                        on_wait=[ws[-1]], on_update=list(si.on_update))
                new.append(inst)
            blk.instructions = new
    return n


def build(Bc=B, Sc=S, x_bufs=8, split_waits=True, q_bufs=2, v_bufs=2, a_bufs=2,
          p_bufs=3, vt_bufs=2, os_bufs=6, dm_bufs=4, mm_bufs=2, sc_bufs=2,
          av_bufs=2, do_attn=True, do_outproj=True):
    """Build the per-core Bass program. Same program for all 8 cores; only
    the weight data differs per core."""
    from contextlib import ExitStack

    KT = D // PT           # 8 contraction tiles
    NCH = Sc // CHUNK      # query chunks per batch
    NTT = Sc // PT         # token/key tiles per batch

    nc = bass.Bass("TRN2", target_bir_lowering=False, debug=False)

    xT_d = nc.dram_tensor("xT", [Bc, D, Sc], f32, kind="ExternalInput")
    wqT_d = nc.dram_tensor("wqT", [D, PT], f32, kind="ExternalInput")
    wkT_d = nc.dram_tensor("wkT", [D, PT], f32, kind="ExternalInput")
    wvT_d = nc.dram_tensor("wvT", [D, PT], f32, kind="ExternalInput")
    woT_d = nc.dram_tensor("woT", [PT, D], f32, kind="ExternalInput")
    id_d = nc.dram_tensor("ident", [PT, PT], f32, kind="ExternalInput")
    mask_d = nc.dram_tensor("mask", [PT, PT], f32, kind="ExternalInput")
    out_d = nc.dram_tensor("outT", [D, Bc * Sc], f32, kind="ExternalOutput")

    with tile.TileContext(nc) as tc, ExitStack() as ctx:
        singles = ctx.enter_context(tc.tile_pool(name="singles", bufs=1))
        pool_x = ctx.enter_context(tc.tile_pool(name="x", bufs=x_bufs))
        pool_q = ctx.enter_context(tc.tile_pool(name="q", bufs=q_bufs * NCH))
        pool_k = ctx.enter_context(tc.tile_pool(name="k", bufs=q_bufs * NCH))
        pool_v = ctx.enter_context(tc.tile_pool(name="v", bufs=v_bufs * NTT))
        pool_a = ctx.enter_context(tc.tile_pool(name="a", bufs=2 * a_bufs))
        pool_vt = ctx.enter_context(tc.tile_pool(name="vt", bufs=vt_bufs))
        pool_p = ctx.enter_context(tc.tile_pool(name="p", bufs=p_bufs))
        pool_dm = ctx.enter_context(tc.tile_pool(name="dm", bufs=dm_bufs))
        pool_os = ctx.enter_context(tc.tile_pool(name="os", bufs=os_bufs))
        ps_mm = ctx.enter_context(tc.tile_pool(name="psmm", bufs=mm_bufs, space="PSUM"))
        ps_sc = ctx.enter_context(tc.tile_pool(name="pssc", bufs=sc_bufs, space="PSUM"))
        ps_av = ctx.enter_context(tc.tile_pool(name="psav", bufs=av_bufs, space="PSUM"))

        # ---- constants ----
        wq_sb = singles.tile([PT, KT, PT], f32r)
        wk_sb = singles.tile([PT, KT, PT], f32r)
        wv_sb = singles.tile([PT, KT, PT], f32r)
        for wsb, wd in ((wq_sb, wqT_d), (wk_sb, wkT_d), (wv_sb, wvT_d)):
            nc.sync.dma_start(
                out=wsb[:, :, :],
                in_=wd.ap().bitcast(f32r).rearrange("(kt p) c -> p kt c", p=PT))
        wo_sb = singles.tile([PT, D], f32r)
        nc.sync.dma_start(out=wo_sb[:, :], in_=woT_d.ap().bitcast(f32r))
        id_sb = singles.tile([PT, PT], f32)
        nc.sync.dma_start(out=id_sb[:, :], in_=id_d.ap())
        mask_sb = singles.tile([PT, PT], f32)
        nc.sync.dma_start(out=mask_sb[:, :], in_=mask_d.ap())

        for b in range(Bc):
            # ---- load xT tiles for this batch ----
            x_sb = []
            for kt in range(KT):
                xt = pool_x.tile([PT, Sc], f32r, tag="x")
                nc.sync.dma_start(out=xt[:, :],
                                  in_=xT_d.ap()[b, kt * PT:(kt + 1) * PT, :].bitcast(f32r))
                x_sb.append(xt)

            # ---- projections (per-chunk tiles for fine-grained deps) ----
            qTs, kTs, v_tiles = [], [], []
            for c in range(NCH):
                cw = slice(c * CHUNK, (c + 1) * CHUNK)
                qc = pool_q.tile([PT, CHUNK], f32r, tag="qT", name=f"q{c}")
                kc = pool_k.tile([PT, CHUNK], f32r, tag="kT", name=f"k{c}")
                qTs.append(qc)
                kTs.append(kc)
                for wsb, dst in ((wq_sb, qc), (wk_sb, kc)):
                    ps = ps_mm.tile([PT, CHUNK], f32, tag="mm")
                    for kt in range(KT):
                        nc.tensor.matmul(ps[:, :], wsb[:, kt, :], x_sb[kt][:, cw],
                                         start=(kt == 0), stop=(kt == KT - 1))
                    nc.scalar.copy(dst[:, :], ps[:, :])
                # v: transposed projection then PE-transpose to natural
                psv = ps_mm.tile([PT, CHUNK], f32, tag="mm")
                for kt in range(KT):
                    nc.tensor.matmul(psv[:, :], wv_sb[:, kt, :], x_sb[kt][:, cw],
                                     start=(kt == 0), stop=(kt == KT - 1))
                vt = pool_vt.tile([PT, CHUNK], f32, tag="vt")
                nc.scalar.copy(vt[:, :], psv[:, :])
                pst = ps_mm.tile([PT, CHUNK], f32, tag="mm")
                for j in range(CHUNK // PT):
                    nc.tensor.transpose(pst[:, j * PT:(j + 1) * PT],
                                        vt[:, j * PT:(j + 1) * PT], id_sb[:, :])
                for j in range(CHUNK // PT):
                    t = c * (CHUNK // PT) + j
                    vtile = pool_v.tile([PT, 256], f32r, tag="v", name=f"v{t}")
                    v_tiles.append(vtile)
                    nc.gpsimd.memset(
                        vtile[:, :].rearrange("p (g x) -> p g x", x=128)
                        [:, :, DK:128].bitcast(u32), 0x3F800000)
                    src = pst[:, j * PT:(j + 1) * PT].rearrange(
                        "p (g x) -> p g x", x=DK)           # [128, 2, 64]
                    dst = vtile[:, :].rearrange(
                        "p (g x) -> p g x", x=128)[:, :, 0:DK]
                    nc.vector.tensor_copy(dst, src)

            # ---- attention, chunk by chunk ----
            aTs = []
            for c in range(NCH if do_attn else 0):
                cw = slice(c * CHUNK, (c + 1) * CHUNK)
                nkt = (c + 1) * (CHUNK // PT)      # causal: key tiles 0..nkt-1
                pso = {}
                for h in (0, 1):
                    pso[h] = ps_av.tile([PT, CHUNK], f32, tag="av", name=f"pso{h}")
                for kt0 in range(0, nkt, 2):
                    for h in (0, 1):
                        hp = slice(h * DK, (h + 1) * DK)
                        pss = ps_sc.tile([PT, 2 * CHUNK], f32, tag="sc")
                        for d in (0, 1):
                            kt = kt0 + d
                            nc.tensor.matmul(
                                pss[:, d * CHUNK:(d + 1) * CHUNK],
                                kTs[kt // (CHUNK // PT)]
                                [hp, (kt % (CHUNK // PT)) * PT:
                                 (kt % (CHUNK // PT) + 1) * PT],
                                qTs[c][hp, :],
                                start=True, stop=True)
                        # causal handling on diagonal key tiles: triangular
                        # additive mask on the partial 128x128 block (DVE);
                        # fully-masked leading columns are never exp'd — the
                        # P region is pre-zeroed on gpsimd off the critical
                        # path and exp covers only the valid column ranges.
                        P = pool_p.tile([PT, 2 * CHUNK], f32r, tag="P")
                        i0 = kt0 - (c * (CHUNK // PT))
                        diag = i0 >= 0
                        if diag:
                            for d, i in ((0, i0), (1, i0 + 1)):
                                if i > 0:
                                    nc.gpsimd.memset(
                                        P[:, d * CHUNK: d * CHUNK + i * PT]
                                        .bitcast(u32), 0)
                            for d, i in ((0, i0), (1, i0 + 1)):
                                off = d * CHUNK
                                nc.vector.tensor_add(
                                    pss[:, off + i * PT: off + (i + 1) * PT],
                                    pss[:, off + i * PT: off + (i + 1) * PT],
                                    mask_sb[:, :])
                                nc.scalar.activation(
                                    out=P[:, off + i * PT:(d + 1) * CHUNK],
                                    in_=pss[:, off + i * PT:(d + 1) * CHUNK],
                                    func=AF.Exp)
                        else:
                            nc.scalar.activation(out=P[:, :], in_=pss[:, :],
                                                 func=AF.Exp)
                        for d in (0, 1):
                            kt = kt0 + d
                            nc.tensor.matmul(
                                pso[h][:, :],
                                v_tiles[kt][:, h * 128:(h + 1) * 128],
                                P[:, d * CHUNK:(d + 1) * CHUNK],
                                start=(kt == 0), stop=(kt == nkt - 1),
                                skip_group_check=True)
                # normalize into the per-chunk stacked aT
                aTc = pool_a.tile([PT, CHUNK], f32r, tag="aT", name=f"aT{c}")
                aTs.append(aTc)
                for h in (0, 1):
                    dm = pool_dm.tile([DK, CHUNK], f32, tag="dm")
                    nc.vector.reciprocal(dm[:, :], pso[h][DK:2 * DK, :])
                    nc.vector.tensor_mul(aTc[h * DK:(h + 1) * DK, :],
                                         pso[h][0:DK, :], dm[:, :])

            # ---- output projection (partial, transposed) ----
            for c in range(NCH if (do_attn and do_outproj) else 0):
                for n in range(D // PT):
                    psp = ps_mm.tile([PT, CHUNK], f32, tag="mm")
                    nc.tensor.matmul(psp[:, :], wo_sb[:, n * PT:(n + 1) * PT],
                                     aTs[c][:, :], start=True, stop=True)
                    ost = pool_os.tile([PT, CHUNK], f32, tag="os")
                    nc.vector.tensor_copy(ost[:, :], psp[:, :])
                    nc.sync.dma_start(
                        out=out_d.ap()[n * PT:(n + 1) * PT,
                                       b * Sc + c * CHUNK:
                                       b * Sc + (c + 1) * CHUNK],
                        in_=ost[:, :])

    if split_waits:
        _split_multi_waits(nc)
    return nc


_build_cache = {}


def _get_program(Bc=B, Sc=S):
    key = (Bc, Sc)
    if key not in _build_cache:
        _build_cache[key] = build(Bc, Sc)
    return _build_cache[key]


def make_in_maps(x, w_q, w_k, w_v, w_o):
    """Host-side sharding: returns per-core input dicts."""
    Bc, Sc, Dc = x.shape
    xT = np.ascontiguousarray(x.transpose(0, 2, 1)).astype(np.float32)
    ident = np.eye(PT, dtype=np.float32)
    jj, qq = np.meshgrid(np.arange(PT), np.arange(PT), indexing="ij")
    mask = np.where(jj <= qq, 0.0, NEG).astype(np.float32)
    scale = DK ** -0.5
    in_maps = []
    for c in range(NCORES):
        rows = slice(PT * c, PT * (c + 1))
        in_maps.append({
            "xT": xT,
            "wqT": np.ascontiguousarray((w_q[rows, :] * scale).T).astype(np.float32),
            "wkT": np.ascontiguousarray(w_k[rows, :].T).astype(np.float32),
            "wvT": np.ascontiguousarray(w_v[rows, :].T).astype(np.float32),
            "woT": np.ascontiguousarray(w_o[:, rows].T).astype(np.float32),
            "ident": ident,
            "mask": mask,
        })
    return in_maps


def run_on_hw(in_maps, Bc=B, Sc=S, trace=False):
    nc = _get_program(Bc, Sc)
    return bass_utils.run_bass_kernel_spmd(
        nc, in_maps, core_ids=list(range(NCORES)), trace=trace)


def kernel(x, w_q, w_k, w_v, w_o, b_o):
    x = np.asarray(x, dtype=np.float32)
    w_q = np.asarray(w_q, dtype=np.float32)
    w_k = np.asarray(w_k, dtype=np.float32)
    w_v = np.asarray(w_v, dtype=np.float32)
    w_o = np.asarray(w_o, dtype=np.float32)
    b_o = np.asarray(b_o, dtype=np.float32)
    Bc, Sc, Dc = x.shape
    in_maps = make_in_maps(x, w_q, w_k, w_v, w_o)
    res = run_on_hw(in_maps, Bc, Sc)
    outT = np.zeros((D, Bc * Sc), dtype=np.float32)
    for c in range(NCORES):
        outT += res.results[c]["outT"]
    out = outT.T.reshape(Bc, Sc, D) + b_o
    return out.astype(np.float32)



# revision 5
# speedup vs baseline: 5.2737x; 5.2737x over previous
"""Causal multi-head attention for Trainium2, 8-core (batch x head-half) parallel.

Problem: B=4, S=2048, D=1024, H=16 heads (dk=64), fp32 in/out.
    q = x @ w_q.T ; k = x @ w_k.T ; v = x @ w_v.T   (per-head split)
    out = softmax(causal(q k^T / 8)) v, concat heads, @ w_o.T + b_o

Sharding: core c owns batch b = c//2 and head-half hh = c%2 (8 heads =
channels [512*hh, 512*hh+512)).  Each core computes q/k/v projections for
its 512 channels over its one batch, runs causal attention for its 8 heads,
and produces a partial output projection outT_c = w_o[:, ch]^T a_c^T of
shape [1024, S]; the host sums core pairs (2b, 2b+1), transposes, adds b_o.

All matmul operands are bf16 (fp32 PSUM accumulation; validated 3.8e-3 max
rel err in numpy vs the 2e-2 gate).  bf16 keeps the PE at 1 cycle/row,
enables FWL fast weight loads, and halves SBUF/DMA vs fp32.

Per-core dataflow (head-pair t = 0..3 maps to SBUF partition tiles):
  - x is pre-transposed + bf16 on host: xT [8, 128, S] so the contraction
    dim D lands on SBUF partitions.
  - projections run weight-stationary kt-outer: one LDWEIGHTS feeds 4
    matmuls (one per 512-token chunk), accumulating in 4 PSUM banks.
  - v is PE-transposed to natural [tok, ch] order and stored as
    [v_h | ones] stationaries: AV then yields both the attention output
    (rows 0-63) and the softmax denominator replicated on rows 64-127.
  - scores are computed transposed (keys on partitions): sT = kT^T qT with
    two heads running concurrently in PE row groups 0-1 / 2-3.
  - softmax without max-subtraction (scores ~N(0,1); exp in fp32 PSUM),
    causal handled by an additive -1e9 triangle mask on exact-diagonal
    128x128 blocks; above-diagonal work inside a diagonal 512-superblock is
    skipped by trimming the matmul free dim (queries < 128*i are never
    computed or exp'd, and the AV accumulation never reads them).
  - normalization: ACT-table reciprocal of the replicated denominator rows
    (8x faster than DVE reciprocal), then one DVE multiply straight into
    the stacked aT layout the output projection consumes.
  - strict phase order (projections -> attention -> output projection)
    keeps the PE densely busy so the HAM clock gate stays at K=8/8.
"""

import numpy as np

import concourse.bass as bass
import concourse.tile as tile
from concourse import mybir
from concourse import bass_utils

f32 = mybir.dt.float32
bf16 = mybir.dt.bfloat16
u32 = mybir.dt.uint32
AF = mybir.ActivationFunctionType

B, S, D, H = 4, 2048, 1024, 16
DK = D // H            # 64
NCORES = 8
PT = 128               # partition tile
CH = 512               # query chunk (PSUM bank = 512 fp32)
KT = D // PT           # 8 contraction tiles over D
T = 4                  # head-pairs per core (8 heads)
NT = D // PT           # 8 output row tiles for the o-projection
NEG = -1.0e9


def _split_multi_waits(nc):
    """This walrus build allows at most one sync-wait per TPB instruction;
    hoist extra waits onto single-wait NoOps on the same engine."""
    n = 0
    for f in nc.m.functions:
        for blk in f.blocks:
            new = []
            for inst in blk.instructions:
                si = inst.sync_info
                if si is not None and si.on_wait and len(si.on_wait) > 1:
                    ws = list(si.on_wait)
                    for w in ws[:-1]:
                        new.append(mybir.InstNoOp(
                            name=f"I-wfix-{n}", ins=[], outs=[], engine=inst.engine,
                            sync_info=mybir.SyncInfo(on_wait=[w], on_update=[])))
                        n += 1
                    inst.sync_info = mybir.SyncInfo(
                        on_wait=[ws[-1]], on_update=list(si.on_update))
                new.append(inst)
            blk.instructions = new
    return n


def build(Sc=S, split_waits=True, p_bufs=6, vt_bufs=3, dm_bufs=4, os_bufs=2,
          acc_bufs=4, att_bufs=4, do_attn=True, do_outproj=True):
    """Build the per-core Bass program. Same program for all 8 cores; only
    the input data differs per core."""
    from contextlib import ExitStack

    NCH = Sc // CH         # query chunks
    NTT = Sc // PT         # token/key tiles

    nc = bass.Bass("TRN2", target_bir_lowering=False, debug=False)

    xT_d = nc.dram_tensor("xT", [KT, PT, Sc], bf16, kind="ExternalInput")
    wq_d = nc.dram_tensor("wq", [PT, KT, T, PT], bf16, kind="ExternalInput")
    wk_d = nc.dram_tensor("wk", [PT, KT, T, PT], bf16, kind="ExternalInput")
    wv_d = nc.dram_tensor("wv", [PT, KT, T, PT], bf16, kind="ExternalInput")
    wo_d = nc.dram_tensor("wo", [PT, T, NT, PT], bf16, kind="ExternalInput")
    id_d = nc.dram_tensor("ident", [PT, PT], bf16, kind="ExternalInput")
    mask_d = nc.dram_tensor("mask", [PT, PT], f32, kind="ExternalInput")
    out_d = nc.dram_tensor("outT", [D, Sc], f32, kind="ExternalOutput")

    with tile.TileContext(nc) as tc, ExitStack() as ctx:
        singles = ctx.enter_context(tc.tile_pool(name="singles", bufs=1))
        pool_P = ctx.enter_context(tc.tile_pool(name="P", bufs=p_bufs))
        pool_vt = ctx.enter_context(tc.tile_pool(name="vt", bufs=vt_bufs))
        pool_dm = ctx.enter_context(tc.tile_pool(name="dm", bufs=dm_bufs))
        pool_os = ctx.enter_context(tc.tile_pool(name="os", bufs=os_bufs))
        ps_acc = ctx.enter_context(tc.tile_pool(name="psacc", bufs=acc_bufs, space="PSUM"))
        ps_att = ctx.enter_context(tc.tile_pool(name="psatt", bufs=att_bufs, space="PSUM"))

        # ---- constants / inputs ----
        wq_sb = singles.tile([PT, KT, T, PT], bf16)
        nc.sync.dma_start(out=wq_sb[:, :, :, :], in_=wq_d.ap())
        x_sb = []
        for kt in range(KT):
            xt = singles.tile([PT, Sc], bf16, name=f"x{kt}")
            nc.sync.dma_start(out=xt[:, :], in_=xT_d.ap()[kt])
            x_sb.append(xt)
        wk_sb = singles.tile([PT, KT, T, PT], bf16)
        nc.sync.dma_start(out=wk_sb[:, :, :, :], in_=wk_d.ap())
        wv_sb = singles.tile([PT, KT, T, PT], bf16)
        nc.sync.dma_start(out=wv_sb[:, :, :, :], in_=wv_d.ap())
        id_sb = singles.tile([PT, PT], bf16)
        nc.sync.dma_start(out=id_sb[:, :], in_=id_d.ap())
        mask_sb = singles.tile([PT, PT], f32)
        nc.sync.dma_start(out=mask_sb[:, :], in_=mask_d.ap())
        wo_sb = singles.tile([PT, T, NT, PT], bf16)
        nc.sync.dma_start(out=wo_sb[:, :, :, :], in_=wo_d.ap())

        qT, kT_sb, v_sb, aT = [], [], [], []
        for t in range(T):
            qt = singles.tile([PT, Sc], bf16, name=f"qT{t}")
            kt_ = singles.tile([PT, Sc], bf16, name=f"kT{t}")
            vt_ = singles.tile([PT, NTT, 2 * PT], bf16, name=f"v{t}")
            at_ = singles.tile([PT, Sc], bf16, name=f"aT{t}")
            qT.append(qt)
            kT_sb.append(kt_)
            v_sb.append(vt_)
            aT.append(at_)
            # ones columns for the [v|1] denominator trick (two bf16 ones
            # per u32). Written once; v copies only touch cols 0:64/128:192.
            nc.gpsimd.memset(
                vt_[:, :, :].rearrange("p g (h x) -> p g h x", x=PT)
                [:, :, :, DK:PT].bitcast(u32), 0x3F803F80)

        # ---- phase 1: projections (weight-stationary kt-outer) ----
        for t in range(T):
            for which, wsb in (("q", wq_sb), ("k", wk_sb), ("v", wv_sb)):
                banks = [ps_acc.tile([PT, CH], f32, tag="acc", name=f"pj{which}{t}{c}")
                         for c in range(NCH)]
                for kt in range(KT):
                    for c in range(NCH):
                        nc.tensor.matmul(
                            banks[c][:, :], wsb[:, kt, t, :],
                            x_sb[kt][:, c * CH:(c + 1) * CH],
                            start=(kt == 0), stop=(kt == KT - 1))
                if which == "q":
                    for c in range(NCH):
                        nc.scalar.copy(qT[t][:, c * CH:(c + 1) * CH], banks[c][:, :])
                elif which == "k":
                    for c in range(NCH):
                        nc.scalar.copy(kT_sb[t][:, c * CH:(c + 1) * CH], banks[c][:, :])
                else:
                    # v: evacuate to SBUF bf16, PE-transpose to natural
                    # [tok, ch] order, interleave into [v_h0|1|v_h1|1].
                    for c in range(NCH):
                        vt = pool_vt.tile([PT, CH], bf16, tag="vt")
                        nc.vector.tensor_copy(vt[:, :], banks[c][:, :])
                        pst = ps_acc.tile([PT, 4, PT], bf16, tag="acc", name=f"tp{t}{c}")
                        for j in range(CH // PT):
                            nc.tensor.transpose(
                                pst[:, j, :], vt[:, j * PT:(j + 1) * PT], id_sb[:, :])
                        src = pst[:, :, :].rearrange("p j (h x) -> p j h x", x=DK)
                        dst = v_sb[t][:, 4 * c:4 * c + 4, :].rearrange(
                            "p j (h x) -> p j h x", x=PT)[:, :, :, 0:DK]
                        nc.vector.tensor_copy(dst, src)

        # ---- phase 2: attention (per head-pair, per query chunk) ----
        for t in range(T if do_attn else 0):
            for c in range(NCH):
                nkt = (c + 1) * (CH // PT)     # causal: key tiles 0..nkt-1
                pso = {}
                for h in (0, 1):
                    pso[h] = ps_att.tile([PT, CH], f32, tag="pso", name=f"pso{t}{c}{h}")
                for kt in range(nkt):
                    i = kt - c * (CH // PT)    # >=0 on the diagonal superblock
                    off = max(i, 0) * PT       # queries < off are fully masked
                    for h in (0, 1):
                        hp = slice(h * DK, (h + 1) * DK)
                        pss = ps_acc.tile([PT, CH], f32, tag="acc", name=f"ss{t}{c}{kt}{h}")
                        nc.tensor.matmul(
                            pss[:, off:CH],
                            kT_sb[t][hp, kt * PT:(kt + 1) * PT],
                            qT[t][hp, c * CH + off:(c + 1) * CH],
                            start=True, stop=True)
                        Pt = pool_P.tile([PT, CH], bf16, tag="P", name=f"P{t}{c}{kt}{h}")
                        if i >= 0:
                            nc.vector.tensor_add(
                                pss[:, off:off + PT], pss[:, off:off + PT],
                                mask_sb[:, :])
                        nc.scalar.activation(
                            out=Pt[:, off:CH], in_=pss[:, off:CH], func=AF.Exp)
                        nc.tensor.matmul(
                            pso[h][:, off:CH],
                            v_sb[t][:, kt, h * PT:(h + 1) * PT],
                            Pt[:, off:CH],
                            start=(kt == 0), stop=(kt == nkt - 1),
                            skip_group_check=True)
                # normalize into the stacked aT layout
                for h in (0, 1):
                    dm = pool_dm.tile([PT, CH], f32, tag="dm")
                    lg = pool_dm.tile([PT, CH], f32, tag="lg")
                    nc.scalar.activation(
                        out=lg[DK:2 * DK, :], in_=pso[h][DK:2 * DK, :],
                        func=AF.Ln)
                    nc.scalar.activation(
                        out=dm[DK:2 * DK, :], in_=lg[DK:2 * DK, :],
                        func=AF.Exp, scale=-1.0)
                    nc.vector.tensor_mul(
                        aT[t][h * DK:(h + 1) * DK, c * CH:(c + 1) * CH],
                        pso[h][0:DK, :], dm[DK:2 * DK, :])

        # ---- phase 3: output projection (partial, transposed) ----
        for nt in range(NT if (do_attn and do_outproj) else 0):
            banks = []
            for c in range(NCH):
                pool = ps_acc if c % 2 == 0 else ps_att
                tag = "acc" if c % 2 == 0 else "pso"
                banks.append(pool.tile([PT, CH], f32, tag=tag, name=f"op{nt}{c}"))
            for ct in range(T):
                for c in range(NCH):
                    nc.tensor.matmul(
                        banks[c][:, :], wo_sb[:, ct, nt, :],
                        aT[ct][:, c * CH:(c + 1) * CH],
                        start=(ct == 0), stop=(ct == T - 1))
            ost = pool_os.tile([PT, Sc], f32, tag="os")
            for c in range(NCH):
                eng = nc.scalar.copy if c % 2 == 0 else nc.vector.tensor_copy
                eng(ost[:, c * CH:(c + 1) * CH], banks[c][:, :])
            nc.sync.dma_start(
                out=out_d.ap()[nt * PT:(nt + 1) * PT, :], in_=ost[:, :])

    if split_waits:
        _split_multi_waits(nc)
    return nc


_build_cache = {}


def _get_program(Sc=S):
    key = Sc
    if key not in _build_cache:
        _build_cache[key] = build(Sc)
    return _build_cache[key]


def _bf16(a):
    import ml_dtypes
    return np.ascontiguousarray(a).astype(ml_dtypes.bfloat16)


def make_in_maps(x, w_q, w_k, w_v, w_o):
    """Host-side sharding: returns per-core input dicts.
    Core c: batch c//2, head-half c%2."""
    Bc, Sc, Dc = x.shape
    scale = DK ** -0.5
    ident = np.eye(PT, dtype=np.float32)
    jj, qq = np.meshgrid(np.arange(PT), np.arange(PT), indexing="ij")
    mask = np.where(jj <= qq, 0.0, NEG).astype(np.float32)

    def pack_w(w):  # [1024, 512] -> [128 p, 8 kt, 4 t, 128 c]
        return np.ascontiguousarray(
            w.reshape(KT, PT, T, PT).transpose(1, 0, 2, 3))

    xTs = [_bf16(x[b].T.reshape(KT, PT, Sc)) for b in range(Bc)]
    whalf = []
    for hh in range(2):
        rows = slice(512 * hh, 512 * hh + 512)
        wo_half = w_o[:, rows].T.reshape(T, PT, NT, PT).transpose(1, 0, 2, 3)
        whalf.append({
            "wq": _bf16(pack_w((w_q[rows, :] * scale).T)),
            "wk": _bf16(pack_w(w_k[rows, :].T)),
            "wv": _bf16(pack_w(w_v[rows, :].T)),
            "wo": _bf16(np.ascontiguousarray(wo_half)),
        })
    in_maps = []
    for c in range(NCORES):
        b, hh = c // 2, c % 2
        m = {"xT": xTs[b], "ident": _bf16(ident), "mask": mask}
        m.update(whalf[hh])
        in_maps.append(m)
    return in_maps


def run_on_hw(in_maps, Sc=S, trace=False, trace_cores=None):
    nc = _get_program(Sc)
    return bass_utils.run_bass_kernel_spmd(
        nc, in_maps, core_ids=list(range(NCORES)), trace=trace,
        trace_cores=trace_cores)


def kernel(x, w_q, w_k, w_v, w_o, b_o):
    x = np.asarray(x, dtype=np.float32)
    w_q = np.asarray(w_q, dtype=np.float32)
    w_k = np.asarray(w_k, dtype=np.float32)
    w_v = np.asarray(w_v, dtype=np.float32)
    w_o = np.asarray(w_o, dtype=np.float32)
    b_o = np.asarray(b_o, dtype=np.float32)
    Bc, Sc, Dc = x.shape
    in_maps = make_in_maps(x, w_q, w_k, w_v, w_o)
    res = run_on_hw(in_maps, Sc)
    out = np.empty((Bc, Sc, Dc), dtype=np.float32)
    for b in range(Bc):
        outT = res.results[2 * b]["outT"] + res.results[2 * b + 1]["outT"]
        out[b] = outT.T + b_o
    return out


# revision 13
# speedup vs baseline: 7.3938x; 1.4020x over previous
"""Causal multi-head attention for Trainium2, 8-core (batch x head-half) parallel.

Problem: B=4, S=2048, D=1024, H=16 heads (dk=64), fp32 in/out.
    q = x @ w_q.T ; k = x @ w_k.T ; v = x @ w_v.T   (per-head split)
    out = softmax(causal(q k^T / 8)) v, concat heads, @ w_o.T + b_o

Sharding: core c owns batch b = c//2 and head-half hh = c%2 (8 heads =
channels [512*hh, 512*hh+512)).  Each core computes q/k/v projections for
its 512 channels over its one batch, runs causal attention for its 8 heads,
and produces a partial output projection outT_c = w_o[:, ch]^T a_c^T of
shape [1024, S]; the host sums core pairs (2b, 2b+1), transposes, adds b_o.

All matmul operands are bf16 (fp32 PSUM accumulation; validated 3.8e-3 max
rel err in numpy vs the 2e-2 gate).  bf16 keeps the PE at 1 cycle/row,
enables FWL fast weight loads, and halves SBUF/DMA vs fp32.

Per-core dataflow (head-pair t = 0..3 maps to SBUF partition tiles):
  - x is pre-transposed + bf16 on host: xT [8, 128, S] so the contraction
    dim D lands on SBUF partitions.
  - projections run weight-stationary kt-outer: one LDWEIGHTS feeds 4
    matmuls (one per 512-token chunk), accumulating in 4 PSUM banks.
  - v is PE-transposed to natural [tok, ch] order and stored as
    [v_h | ones] stationaries: AV then yields both the attention output
    (rows 0-63) and the softmax denominator replicated on rows 64-127.
  - scores are computed transposed (keys on partitions): sT = kT^T qT with
    two heads running concurrently in PE row groups 0-1 / 2-3.
  - softmax without max-subtraction (scores ~N(0,1); exp in fp32 PSUM),
    causal handled by an additive -1e9 triangle mask on exact-diagonal
    128x128 blocks; above-diagonal work inside a diagonal 512-superblock is
    skipped by trimming the matmul free dim (queries < 128*i are never
    computed or exp'd, and the AV accumulation never reads them).
  - normalization: ACT-table reciprocal of the replicated denominator rows
    (8x faster than DVE reciprocal), then one DVE multiply straight into
    the stacked aT layout the output projection consumes.
  - strict phase order (projections -> attention -> output projection)
    keeps the PE densely busy so the HAM clock gate stays at K=8/8.
"""

import numpy as np

import concourse.bass as bass
import concourse.tile as tile
from concourse import mybir
from concourse import bass_utils

f32 = mybir.dt.float32
bf16 = mybir.dt.bfloat16
u32 = mybir.dt.uint32
AF = mybir.ActivationFunctionType

B, S, D, H = 4, 2048, 1024, 16
DK = D // H            # 64
NCORES = 8
PT = 128               # partition tile
CH = 512               # query chunk (PSUM bank = 512 fp32)
KT = D // PT           # 8 contraction tiles over D
T = 4                  # head-pairs per core (8 heads)
NT = D // PT           # 8 output row tiles for the o-projection
NEG = -1.0e9


def _split_multi_waits(nc):
    """This walrus build allows at most one sync-wait per TPB instruction;
    hoist extra waits onto single-wait NoOps on the same engine."""
    n = 0
    for f in nc.m.functions:
        for blk in f.blocks:
            new = []
            for inst in blk.instructions:
                si = inst.sync_info
                if si is not None and si.on_wait and len(si.on_wait) > 1:
                    ws = list(si.on_wait)
                    for w in ws[:-1]:
                        new.append(mybir.InstNoOp(
                            name=f"I-wfix-{n}", ins=[], outs=[], engine=inst.engine,
                            sync_info=mybir.SyncInfo(on_wait=[w], on_update=[])))
                        n += 1
                    inst.sync_info = mybir.SyncInfo(
                        on_wait=[ws[-1]], on_update=list(si.on_update))
                new.append(inst)
            blk.instructions = new
    return n


def build(Sc=S, split_waits=True, p_bufs=6, vt_bufs=3, dm_bufs=2, nm_bufs=4,
          os_bufs=2, acc_bufs=3, att_bufs=2, do_attn=True, do_outproj=True):
    """Build the per-core Bass program. Same program for all 8 cores; only
    the input data differs per core."""
    from contextlib import ExitStack

    NCH = Sc // CH         # query chunks
    NTT = Sc // PT         # token/key tiles

    nc = bass.Bass("TRN2", target_bir_lowering=False, debug=False)

    xT_d = nc.dram_tensor("xT", [KT, PT, Sc], bf16, kind="ExternalInput")
    wq_d = nc.dram_tensor("wq", [PT, KT, T, PT], bf16, kind="ExternalInput")
    wk_d = nc.dram_tensor("wk", [PT, KT, T, PT], bf16, kind="ExternalInput")
    wv_d = nc.dram_tensor("wv", [PT, KT, T, PT], bf16, kind="ExternalInput")
    wo_d = nc.dram_tensor("wo", [PT, T, NT, PT], bf16, kind="ExternalInput")
    id_d = nc.dram_tensor("ident", [PT, PT], bf16, kind="ExternalInput")
    mask_d = nc.dram_tensor("mask", [PT, PT], f32, kind="ExternalInput")
    out_d = nc.dram_tensor("outT", [D, Sc], f32, kind="ExternalOutput")

    with tile.TileContext(nc) as tc, ExitStack() as ctx:
        singles = ctx.enter_context(tc.tile_pool(name="singles", bufs=1))
        pool_P = ctx.enter_context(tc.tile_pool(name="P", bufs=p_bufs))
        pool_vt = ctx.enter_context(tc.tile_pool(name="vt", bufs=vt_bufs))
        pool_dm = ctx.enter_context(tc.tile_pool(name="dm", bufs=dm_bufs))
        pool_nm = ctx.enter_context(tc.tile_pool(name="nm", bufs=nm_bufs))
        pool_os = ctx.enter_context(tc.tile_pool(name="os", bufs=os_bufs))
        ps_acc = ctx.enter_context(tc.tile_pool(name="psacc", bufs=acc_bufs, space="PSUM"))
        ps_att = ctx.enter_context(tc.tile_pool(name="psatt", bufs=att_bufs, space="PSUM"))

        # ---- constants / inputs ----
        wq_sb = singles.tile([PT, KT, T, PT], bf16)
        nc.sync.dma_start(out=wq_sb[:, :, :, :], in_=wq_d.ap())
        x_sb = []
        for kt in range(KT):
            xt = singles.tile([PT, Sc], bf16, name=f"x{kt}")
            nc.sync.dma_start(out=xt[:, :], in_=xT_d.ap()[kt])
            x_sb.append(xt)
        wk_sb = singles.tile([PT, KT, T, PT], bf16)
        nc.sync.dma_start(out=wk_sb[:, :, :, :], in_=wk_d.ap())
        wv_sb = singles.tile([PT, KT, T, PT], bf16)
        nc.sync.dma_start(out=wv_sb[:, :, :, :], in_=wv_d.ap())
        id_sb = singles.tile([PT, PT], bf16)
        nc.sync.dma_start(out=id_sb[:, :], in_=id_d.ap())
        mask_sb = singles.tile([PT, PT], f32)
        nc.sync.dma_start(out=mask_sb[:, :], in_=mask_d.ap())
        wo_sb = singles.tile([PT, T, NT, PT], bf16)
        nc.sync.dma_start(out=wo_sb[:, :, :, :], in_=wo_d.ap())

        qT, kT_sb, v_sb, aT = [], [], [], []
        for t in range(T):
            qt = singles.tile([PT, Sc], bf16, name=f"qT{t}")
            kt_ = singles.tile([PT, Sc], bf16, name=f"kT{t}")
            vt_ = singles.tile([PT, NTT, 2 * PT], bf16, name=f"v{t}")
            at_ = singles.tile([PT, Sc], bf16, name=f"aT{t}")
            qT.append(qt)
            kT_sb.append(kt_)
            v_sb.append(vt_)
            aT.append(at_)
            # ones columns for the [v|1] denominator trick (two bf16 ones
            # per u32). Written once; v copies only touch cols 0:64/128:192.
            nc.gpsimd.memset(
                vt_[:, :, :].rearrange("p g (h x) -> p g h x", x=PT)
                [:, :, :, DK:PT].bitcast(u32), 0x3F803F80)

        # ---- phase 1: projections (weight-stationary kt-outer) ----
        # PSUM "acc" slots are 2-bank [PT, 2, CH] tiles; each projection
        # group accumulates its NCH=4 chunk banks in 2 tiles.
        for t in range(T):
            for which, wsb in (("q", wq_sb), ("k", wk_sb), ("v", wv_sb)):
                bt = [ps_acc.tile([PT, 2, CH], f32, tag="acc", name=f"pj{which}{t}{g}")
                      for g in range((NCH + 1) // 2)]
                banks = [bt[c // 2][:, c % 2, :] for c in range(NCH)]
                for kt in range(KT):
                    for c in range(NCH):
                        nc.tensor.matmul(
                            banks[c], wsb[:, kt, t, :],
                            x_sb[kt][:, c * CH:(c + 1) * CH],
                            start=(kt == 0), stop=(kt == KT - 1))
                if which == "q":
                    for c in range(NCH):
                        nc.scalar.copy(qT[t][:, c * CH:(c + 1) * CH], banks[c])
                elif which == "k":
                    for c in range(NCH):
                        nc.scalar.copy(kT_sb[t][:, c * CH:(c + 1) * CH], banks[c])
                else:
                    # v: evacuate to SBUF bf16, PE-transpose to natural
                    # [tok, ch] order, interleave into [v_h0|1|v_h1|1].
                    for c in range(NCH):
                        vt = pool_vt.tile([PT, CH], bf16, tag="vt")
                        nc.vector.tensor_copy(vt[:, :], banks[c])
                        pst = ps_acc.tile([PT, 4, PT], bf16, tag="acc", name=f"tp{t}{c}")
                        for j in range(CH // PT):
                            nc.tensor.transpose(
                                pst[:, j, :], vt[:, j * PT:(j + 1) * PT], id_sb[:, :])
                        src = pst[:, :, :].rearrange("p j (h x) -> p j h x", x=DK)
                        dst = v_sb[t][:, 4 * c:4 * c + 4, :].rearrange(
                            "p j (h x) -> p j h x", x=PT)[:, :, :, 0:DK]
                        nc.vector.tensor_copy(dst, src)

        # ---- phase 2: attention (per head-pair, per query chunk) ----
        # Software-pipelined: scores+exp for key-tile kt+L are emitted before
        # the AV matmuls of key-tile kt, so the PE never waits on the ACT
        # engine's exp and the HAM clock gate stays at K=8/8.  Both heads'
        # scores live in one 2-bank pss tile and share one exp instruction.
        LA = acc_bufs  # scores lookahead (kt units) = pss slot count
        for t in range(T if do_attn else 0):
            for c in range(NCH):
                nkt = (c + 1) * (CH // PT)     # causal: key tiles 0..nkt-1
                pso = {}
                for h in (0, 1):
                    pso[h] = ps_att.tile([PT, CH], f32, tag="pso", name=f"pso{t}{c}{h}")

                def emit_S(kt, t=t, c=c):
                    i = kt - c * (CH // PT)    # >=0 on the diagonal superblock
                    off = max(i, 0) * PT       # queries < off are fully masked
                    pss = ps_acc.tile([PT, 2, CH], f32, tag="acc", name=f"ss{t}{c}{kt}")
                    for h in (0, 1):
                        hp = slice(h * DK, (h + 1) * DK)
                        nc.tensor.matmul(
                            pss[:, h, off:CH],
                            kT_sb[t][hp, kt * PT:(kt + 1) * PT],
                            qT[t][hp, c * CH + off:(c + 1) * CH],
                            start=True, stop=True)
                    if i >= 0:
                        for h in (0, 1):
                            nc.vector.tensor_add(
                                pss[:, h, off:off + PT], pss[:, h, off:off + PT],
                                mask_sb[:, :])
                    Pt = pool_P.tile([PT, 2, CH], bf16, tag="P", name=f"P{t}{c}{kt}")
                    nc.scalar.activation(
                        out=Pt[:, :, off:CH], in_=pss[:, :, off:CH], func=AF.Exp)
                    return Pt, off

                def emit_A(kt, Pt, off, t=t, c=c, nkt=nkt, pso=pso):
                    for h in (0, 1):
                        nc.tensor.matmul(
                            pso[h][:, off:CH],
                            v_sb[t][:, kt, h * PT:(h + 1) * PT],
                            Pt[:, h, off:CH],
                            start=(kt == 0), stop=(kt == nkt - 1),
                            skip_group_check=True)

                steps = {}
                for j in range(min(LA, nkt)):
                    steps[j] = emit_S(j)
                for kt in range(nkt):
                    if kt + LA < nkt:
                        steps[kt + LA] = emit_S(kt + LA)
                    emit_A(kt, *steps.pop(kt))

                # normalize into the stacked aT layout (copy-first so the
                # pso banks free immediately for the next chunk)
                for h in (0, 1):
                    # copy-first (frees the pso banks), shifting the
                    # denominator rows to base partition 0 on the way so the
                    # whole SBUF norm chain is same-base (verifier rule).
                    nm = pool_nm.tile([PT, CH], f32, tag="nm")
                    dn = pool_dm.tile([PT, CH], f32, tag="dn")
                    lg = pool_dm.tile([PT, CH], f32, tag="lg")
                    dm = pool_dm.tile([PT, CH], f32, tag="dm")
                    nc.vector.tensor_copy(nm[0:DK, :], pso[h][0:DK, :])
                    nc.vector.tensor_copy(dn[0:DK, :], pso[h][DK:2 * DK, :])
                    nc.scalar.activation(
                        out=lg[0:DK, :], in_=dn[0:DK, :], func=AF.Ln)
                    nc.scalar.activation(
                        out=dm[0:DK, :], in_=lg[0:DK, :], func=AF.Exp, scale=-1.0)
                    nc.vector.tensor_mul(
                        aT[t][h * DK:(h + 1) * DK, c * CH:(c + 1) * CH],
                        nm[0:DK, :], dm[0:DK, :])

        # ---- phase 3: output projection (partial, transposed) ----
        for nt in range(NT if (do_attn and do_outproj) else 0):
            bt = [ps_acc.tile([PT, 2, CH], f32, tag="acc", name=f"op{nt}{g}")
                  for g in range((NCH + 1) // 2)]
            banks = [bt[c // 2][:, c % 2, :] for c in range(NCH)]
            for ct in range(T):
                for c in range(NCH):
                    nc.tensor.matmul(
                        banks[c], wo_sb[:, ct, nt, :],
                        aT[ct][:, c * CH:(c + 1) * CH],
                        start=(ct == 0), stop=(ct == T - 1))
            ost = pool_os.tile([PT, Sc], f32, tag="os")
            for c in range(NCH):
                eng = nc.scalar.copy if c % 2 == 0 else nc.vector.tensor_copy
                eng(ost[:, c * CH:(c + 1) * CH], banks[c])
            nc.sync.dma_start(
                out=out_d.ap()[nt * PT:(nt + 1) * PT, :], in_=ost[:, :])

    if split_waits:
        _split_multi_waits(nc)
    return nc


_build_cache = {}


def _get_program(Sc=S):
    key = Sc
    if key not in _build_cache:
        _build_cache[key] = build(Sc)
    return _build_cache[key]


def _bf16(a):
    import ml_dtypes
    return np.ascontiguousarray(a).astype(ml_dtypes.bfloat16)


def make_in_maps(x, w_q, w_k, w_v, w_o):
    """Host-side sharding: returns per-core input dicts.
    Core c: batch c//2, head-half c%2."""
    Bc, Sc, Dc = x.shape
    scale = DK ** -0.5
    ident = np.eye(PT, dtype=np.float32)
    jj, qq = np.meshgrid(np.arange(PT), np.arange(PT), indexing="ij")
    mask = np.where(jj <= qq, 0.0, NEG).astype(np.float32)

    def pack_w(w):  # [1024, 512] -> [128 p, 8 kt, 4 t, 128 c]
        return np.ascontiguousarray(
            w.reshape(KT, PT, T, PT).transpose(1, 0, 2, 3))

    xTs = [_bf16(x[b].T.reshape(KT, PT, Sc)) for b in range(Bc)]
    whalf = []
    for hh in range(2):
        rows = slice(512 * hh, 512 * hh + 512)
        wo_half = w_o[:, rows].T.reshape(T, PT, NT, PT).transpose(1, 0, 2, 3)
        whalf.append({
            "wq": _bf16(pack_w((w_q[rows, :] * scale).T)),
            "wk": _bf16(pack_w(w_k[rows, :].T)),
            "wv": _bf16(pack_w(w_v[rows, :].T)),
            "wo": _bf16(np.ascontiguousarray(wo_half)),
        })
    in_maps = []
    for c in range(NCORES):
        b, hh = c // 2, c % 2
        m = {"xT": xTs[b], "ident": _bf16(ident), "mask": mask}
        m.update(whalf[hh])
        in_maps.append(m)
    return in_maps


def run_on_hw(in_maps, Sc=S, trace=False, trace_cores=None):
    nc = _get_program(Sc)
    return bass_utils.run_bass_kernel_spmd(
        nc, in_maps, core_ids=list(range(NCORES)), trace=trace,
        trace_cores=trace_cores)


def kernel(x, w_q, w_k, w_v, w_o, b_o):
    x = np.asarray(x, dtype=np.float32)
    w_q = np.asarray(w_q, dtype=np.float32)
    w_k = np.asarray(w_k, dtype=np.float32)
    w_v = np.asarray(w_v, dtype=np.float32)
    w_o = np.asarray(w_o, dtype=np.float32)
    b_o = np.asarray(b_o, dtype=np.float32)
    Bc, Sc, Dc = x.shape
    in_maps = make_in_maps(x, w_q, w_k, w_v, w_o)
    res = run_on_hw(in_maps, Sc)
    out = np.empty((Bc, Sc, Dc), dtype=np.float32)
    for b in range(Bc):
        outT = res.results[2 * b]["outT"] + res.results[2 * b + 1]["outT"]
        out[b] = outT.T + b_o
    return out


# revision 15
# speedup vs baseline: 7.9084x; 1.0696x over previous
"""Causal multi-head attention for Trainium2, 8-core (batch x head-half) parallel.

Problem: B=4, S=2048, D=1024, H=16 heads (dk=64), fp32 in/out.
    q = x @ w_q.T ; k = x @ w_k.T ; v = x @ w_v.T   (per-head split)
    out = softmax(causal(q k^T / 8)) v, concat heads, @ w_o.T + b_o

Sharding: core c owns batch b = c//2 and head-half hh = c%2 (8 heads =
channels [512*hh, 512*hh+512)).  Each core computes q/k/v projections for
its 512 channels over its one batch, runs causal attention for its 8 heads,
and produces a partial output projection outT_c = w_o[:, ch]^T a_c^T of
shape [1024, S]; the host sums core pairs (2b, 2b+1), transposes, adds b_o.

All matmul operands are bf16 (fp32 PSUM accumulation; validated 3.8e-3 max
rel err in numpy vs the 2e-2 gate).  bf16 keeps the PE at 1 cycle/row,
enables FWL fast weight loads, and halves SBUF/DMA vs fp32.

Per-core dataflow (head-pair t = 0..3 maps to SBUF partition tiles):
  - x is pre-transposed + bf16 on host: xT [8, 128, S] so the contraction
    dim D lands on SBUF partitions.
  - projections run weight-stationary kt-outer: one LDWEIGHTS feeds 4
    matmuls (one per 512-token chunk), accumulating in 4 PSUM banks.
  - v is PE-transposed to natural [tok, ch] order and stored as
    [v_h | ones] stationaries: AV then yields both the attention output
    (rows 0-63) and the softmax denominator replicated on rows 64-127.
  - scores are computed transposed (keys on partitions): sT = kT^T qT with
    two heads running concurrently in PE row groups 0-1 / 2-3.
  - softmax without max-subtraction (scores ~N(0,1); exp in fp32 PSUM),
    causal handled by an additive -1e9 triangle mask on exact-diagonal
    128x128 blocks; above-diagonal work inside a diagonal 512-superblock is
    skipped by trimming the matmul free dim (queries < 128*i are never
    computed or exp'd, and the AV accumulation never reads them).
  - normalization: ACT-table reciprocal of the replicated denominator rows
    (8x faster than DVE reciprocal), then one DVE multiply straight into
    the stacked aT layout the output projection consumes.
  - strict phase order (projections -> attention -> output projection)
    keeps the PE densely busy so the HAM clock gate stays at K=8/8.
"""

import numpy as np

import concourse.bass as bass
import concourse.tile as tile
from concourse import mybir
from concourse import bass_utils

f32 = mybir.dt.float32
bf16 = mybir.dt.bfloat16
u32 = mybir.dt.uint32
AF = mybir.ActivationFunctionType

B, S, D, H = 4, 2048, 1024, 16
DK = D // H            # 64
NCORES = 8
PT = 128               # partition tile
CH = 512               # query chunk (PSUM bank = 512 fp32)
KT = D // PT           # 8 contraction tiles over D
T = 4                  # head-pairs per core (8 heads)
NT = D // PT           # 8 output row tiles for the o-projection
NEG = -1.0e9


def _split_multi_waits(nc):
    """This walrus build allows at most one sync-wait per TPB instruction;
    hoist extra waits onto single-wait NoOps on the same engine."""
    n = 0
    for f in nc.m.functions:
        for blk in f.blocks:
            new = []
            for inst in blk.instructions:
                si = inst.sync_info
                if si is not None and si.on_wait and len(si.on_wait) > 1:
                    ws = list(si.on_wait)
                    for w in ws[:-1]:
                        new.append(mybir.InstNoOp(
                            name=f"I-wfix-{n}", ins=[], outs=[], engine=inst.engine,
                            sync_info=mybir.SyncInfo(on_wait=[w], on_update=[])))
                        n += 1
                    inst.sync_info = mybir.SyncInfo(
                        on_wait=[ws[-1]], on_update=list(si.on_update))
                new.append(inst)
            blk.instructions = new
    return n


def build(Sc=S, split_waits=True, p_bufs=6, vt_bufs=3, dm_bufs=2, nm_bufs=4,
          os_bufs=2, acc_bufs=3, att_bufs=2, do_attn=True, do_outproj=True):
    """Build the per-core Bass program. Same program for all 8 cores; only
    the input data differs per core."""
    from contextlib import ExitStack

    NCH = Sc // CH         # query chunks
    NTT = Sc // PT         # token/key tiles

    nc = bass.Bass("TRN2", target_bir_lowering=False, debug=False)

    xT_d = nc.dram_tensor("xT", [KT, PT, Sc], bf16, kind="ExternalInput")
    wq_d = nc.dram_tensor("wq", [PT, KT, T, PT], bf16, kind="ExternalInput")
    wk_d = nc.dram_tensor("wk", [PT, KT, T, PT], bf16, kind="ExternalInput")
    wv_d = nc.dram_tensor("wv", [PT, KT, T, PT], bf16, kind="ExternalInput")
    wo_d = nc.dram_tensor("wo", [PT, T, NT, PT], bf16, kind="ExternalInput")
    id_d = nc.dram_tensor("ident", [PT, PT], bf16, kind="ExternalInput")
    mask_d = nc.dram_tensor("mask", [PT, PT], f32, kind="ExternalInput")
    out_d = nc.dram_tensor("outT", [D, Sc], f32, kind="ExternalOutput")

    with tile.TileContext(nc) as tc, ExitStack() as ctx:
        singles = ctx.enter_context(tc.tile_pool(name="singles", bufs=1))
        pool_P = ctx.enter_context(tc.tile_pool(name="P", bufs=p_bufs))
        pool_vt = ctx.enter_context(tc.tile_pool(name="vt", bufs=vt_bufs))
        pool_dm = ctx.enter_context(tc.tile_pool(name="dm", bufs=dm_bufs))
        pool_nm = ctx.enter_context(tc.tile_pool(name="nm", bufs=nm_bufs))
        pool_os = ctx.enter_context(tc.tile_pool(name="os", bufs=os_bufs))
        ps_acc = ctx.enter_context(tc.tile_pool(name="psacc", bufs=acc_bufs, space="PSUM"))
        ps_att = ctx.enter_context(tc.tile_pool(name="psatt", bufs=att_bufs, space="PSUM"))

        # ---- constants / inputs ----
        wq_sb = singles.tile([PT, KT, T, PT], bf16)
        nc.sync.dma_start(out=wq_sb[:, :, :, :], in_=wq_d.ap())
        x_sb = []
        for kt in range(KT):
            xt = singles.tile([PT, Sc], bf16, name=f"x{kt}")
            nc.sync.dma_start(out=xt[:, :], in_=xT_d.ap()[kt])
            x_sb.append(xt)
        wk_sb = singles.tile([PT, KT, T, PT], bf16)
        nc.sync.dma_start(out=wk_sb[:, :, :, :], in_=wk_d.ap())
        wv_sb = singles.tile([PT, KT, T, PT], bf16)
        nc.sync.dma_start(out=wv_sb[:, :, :, :], in_=wv_d.ap())
        id_sb = singles.tile([PT, PT], bf16)
        nc.sync.dma_start(out=id_sb[:, :], in_=id_d.ap())
        mask_sb = singles.tile([PT, PT], f32)
        nc.sync.dma_start(out=mask_sb[:, :], in_=mask_d.ap())
        wo_sb = singles.tile([PT, T, NT, PT], bf16)
        nc.sync.dma_start(out=wo_sb[:, :, :, :], in_=wo_d.ap())

        qT, kT_sb, v_sb, aT = [], [], [], []
        for t in range(T):
            qt = singles.tile([PT, Sc], bf16, name=f"qT{t}")
            kt_ = singles.tile([PT, Sc], bf16, name=f"kT{t}")
            vt_ = singles.tile([PT, NTT, 2 * PT], bf16, name=f"v{t}")
            at_ = singles.tile([PT, Sc], bf16, name=f"aT{t}")
            qT.append(qt)
            kT_sb.append(kt_)
            v_sb.append(vt_)
            aT.append(at_)
            # ones columns for the [v|1] denominator trick (two bf16 ones
            # per u32). Written once; v copies only touch cols 0:64/128:192.
            nc.gpsimd.memset(
                vt_[:, :, :].rearrange("p g (h x) -> p g h x", x=PT)
                [:, :, :, DK:PT].bitcast(u32), 0x3F803F80)

        # ---- phase 1: projections (weight-stationary kt-outer) ----
        # PSUM "acc" slots are 2-bank [PT, 2, CH] tiles; each projection
        # group accumulates its NCH=4 chunk banks in 2 tiles.
        for t in range(T):
            for which, wsb in (("q", wq_sb), ("k", wk_sb), ("v", wv_sb)):
                bt = [ps_acc.tile([PT, 2, CH], f32, tag="acc", name=f"pj{which}{t}{g}")
                      for g in range((NCH + 1) // 2)]
                banks = [bt[c // 2][:, c % 2, :] for c in range(NCH)]
                for kt in range(KT):
                    for c in range(NCH):
                        nc.tensor.matmul(
                            banks[c], wsb[:, kt, t, :],
                            x_sb[kt][:, c * CH:(c + 1) * CH],
                            start=(kt == 0), stop=(kt == KT - 1))
                if which == "q":
                    for c in range(NCH):
                        nc.scalar.copy(qT[t][:, c * CH:(c + 1) * CH], banks[c])
                elif which == "k":
                    for c in range(NCH):
                        nc.scalar.copy(kT_sb[t][:, c * CH:(c + 1) * CH], banks[c])
                else:
                    # v: evacuate to SBUF bf16, PE-transpose to natural
                    # [tok, ch] order, interleave into [v_h0|1|v_h1|1].
                    for c in range(NCH):
                        vt = pool_vt.tile([PT, CH], bf16, tag="vt")
                        nc.vector.tensor_copy(vt[:, :], banks[c])
                        pst = ps_acc.tile([PT, 4, PT], bf16, tag="acc", name=f"tp{t}{c}")
                        for j in range(CH // PT):
                            nc.tensor.transpose(
                                pst[:, j, :], vt[:, j * PT:(j + 1) * PT], id_sb[:, :])
                        src = pst[:, :, :].rearrange("p j (h x) -> p j h x", x=DK)
                        dst = v_sb[t][:, 4 * c:4 * c + 4, :].rearrange(
                            "p j (h x) -> p j h x", x=PT)[:, :, :, 0:DK]
                        nc.vector.tensor_copy(dst, src)

        # ---- phase 2: attention (per head-pair, per query chunk) ----
        # Software-pipelined: scores+exp for key-tile kt+L are emitted before
        # the AV matmuls of key-tile kt, so the PE never waits on the ACT
        # engine's exp and the HAM clock gate stays at K=8/8.  Both heads'
        # scores live in one 2-bank pss tile and share one exp instruction.
        LA = acc_bufs  # scores lookahead (kt units) = pss slot count
        pend_norm = []
        for t in range(T if do_attn else 0):
            for c in range(NCH):
                nkt = (c + 1) * (CH // PT)     # causal: key tiles 0..nkt-1
                pso = {}
                for h in (0, 1):
                    pso[h] = ps_att.tile([PT, CH], f32, tag="pso", name=f"pso{t}{c}{h}")

                def emit_S(kt, t=t, c=c):
                    i = kt - c * (CH // PT)    # >=0 on the diagonal superblock
                    off = max(i, 0) * PT       # queries < off are fully masked
                    pss = ps_acc.tile([PT, 2, CH], f32, tag="acc", name=f"ss{t}{c}{kt}")
                    for h in (0, 1):
                        hp = slice(h * DK, (h + 1) * DK)
                        nc.tensor.matmul(
                            pss[:, h, off:CH],
                            kT_sb[t][hp, kt * PT:(kt + 1) * PT],
                            qT[t][hp, c * CH + off:(c + 1) * CH],
                            start=True, stop=True)
                    if i >= 0:
                        for h in (0, 1):
                            nc.vector.tensor_add(
                                pss[:, h, off:off + PT], pss[:, h, off:off + PT],
                                mask_sb[:, :])
                    Pt = pool_P.tile([PT, 2, CH], bf16, tag="P", name=f"P{t}{c}{kt}")
                    nc.scalar.activation(
                        out=Pt[:, :, off:CH], in_=pss[:, :, off:CH], func=AF.Exp)
                    return Pt, off

                def emit_A(kt, Pt, off, t=t, c=c, nkt=nkt, pso=pso):
                    for h in (0, 1):
                        nc.tensor.matmul(
                            pso[h][:, off:CH],
                            v_sb[t][:, kt, h * PT:(h + 1) * PT],
                            Pt[:, h, off:CH],
                            start=(kt == 0), stop=(kt == nkt - 1),
                            skip_group_check=True)

                steps = {}
                for j in range(min(LA, nkt)):
                    steps[j] = emit_S(j)
                # previous chunk's deferred normalization goes here, after
                # this chunk's score prologue, so its ACT ops never sit in
                # front of the exp the PE is about to wait on
                if pend_norm:
                    pend_norm.pop(0)()
                for kt in range(nkt):
                    if kt + LA < nkt:
                        steps[kt + LA] = emit_S(kt + LA)
                    emit_A(kt, *steps.pop(kt))

                def emit_norm(t=t, c=c, pso=pso):
                    # copy-first (frees the pso banks fast), stacking both
                    # heads' numerators/denominators into [128, CH] tiles so
                    # one ln+exp+mul serves the whole chunk.  The PSUM->SBUF
                    # copies also shift the denominator rows so every SBUF op
                    # is same-base (verifier rule).
                    nm = pool_nm.tile([PT, CH], f32, tag="nm")
                    dn = pool_dm.tile([PT, CH], f32, tag="dn")
                    lg = pool_dm.tile([PT, CH], f32, tag="lg")
                    dm = pool_dm.tile([PT, CH], f32, tag="dm")
                    for h in (0, 1):
                        hr = slice(h * DK, (h + 1) * DK)
                        nc.vector.tensor_copy(nm[hr, :], pso[h][0:DK, :])
                        nc.vector.tensor_copy(dn[hr, :], pso[h][DK:2 * DK, :])
                    nc.scalar.activation(out=lg[:, :], in_=dn[:, :], func=AF.Ln)
                    nc.scalar.activation(out=dm[:, :], in_=lg[:, :],
                                         func=AF.Exp, scale=-1.0)
                    nc.vector.tensor_mul(
                        aT[t][:, c * CH:(c + 1) * CH], nm[:, :], dm[:, :])

                pend_norm.append(emit_norm)
        for fn in (pend_norm if do_attn else []):
            fn()

        # ---- phase 3: output projection (partial, transposed) ----
        for nt in range(NT if (do_attn and do_outproj) else 0):
            bt = [ps_acc.tile([PT, 2, CH], f32, tag="acc", name=f"op{nt}{g}")
                  for g in range((NCH + 1) // 2)]
            banks = [bt[c // 2][:, c % 2, :] for c in range(NCH)]
            for ct in range(T):
                for c in range(NCH):
                    nc.tensor.matmul(
                        banks[c], wo_sb[:, ct, nt, :],
                        aT[ct][:, c * CH:(c + 1) * CH],
                        start=(ct == 0), stop=(ct == T - 1))
            ost = pool_os.tile([PT, Sc], f32, tag="os")
            for c in range(NCH):
                eng = nc.scalar.copy if c % 2 == 0 else nc.vector.tensor_copy
                eng(ost[:, c * CH:(c + 1) * CH], banks[c])
            nc.sync.dma_start(
                out=out_d.ap()[nt * PT:(nt + 1) * PT, :], in_=ost[:, :])

    if split_waits:
        _split_multi_waits(nc)
    return nc


_build_cache = {}


def _get_program(Sc=S):
    key = Sc
    if key not in _build_cache:
        _build_cache[key] = build(Sc)
    return _build_cache[key]


def _bf16(a):
    import ml_dtypes
    return np.ascontiguousarray(a).astype(ml_dtypes.bfloat16)


def make_in_maps(x, w_q, w_k, w_v, w_o):
    """Host-side sharding: returns per-core input dicts.
    Core c: batch c//2, head-half c%2."""
    Bc, Sc, Dc = x.shape
    scale = DK ** -0.5
    ident = np.eye(PT, dtype=np.float32)
    jj, qq = np.meshgrid(np.arange(PT), np.arange(PT), indexing="ij")
    mask = np.where(jj <= qq, 0.0, NEG).astype(np.float32)

    def pack_w(w):  # [1024, 512] -> [128 p, 8 kt, 4 t, 128 c]
        return np.ascontiguousarray(
            w.reshape(KT, PT, T, PT).transpose(1, 0, 2, 3))

    xTs = [_bf16(x[b].T.reshape(KT, PT, Sc)) for b in range(Bc)]
    whalf = []
    for hh in range(2):
        rows = slice(512 * hh, 512 * hh + 512)
        wo_half = w_o[:, rows].T.reshape(T, PT, NT, PT).transpose(1, 0, 2, 3)
        whalf.append({
            "wq": _bf16(pack_w((w_q[rows, :] * scale).T)),
            "wk": _bf16(pack_w(w_k[rows, :].T)),
            "wv": _bf16(pack_w(w_v[rows, :].T)),
            "wo": _bf16(np.ascontiguousarray(wo_half)),
        })
    in_maps = []
    for c in range(NCORES):
        b, hh = c // 2, c % 2
        m = {"xT": xTs[b], "ident": _bf16(ident), "mask": mask}
        m.update(whalf[hh])
        in_maps.append(m)
    return in_maps


def run_on_hw(in_maps, Sc=S, trace=False, trace_cores=None):
    nc = _get_program(Sc)
    return bass_utils.run_bass_kernel_spmd(
        nc, in_maps, core_ids=list(range(NCORES)), trace=trace,
        trace_cores=trace_cores)


def kernel(x, w_q, w_k, w_v, w_o, b_o):
    x = np.asarray(x, dtype=np.float32)
    w_q = np.asarray(w_q, dtype=np.float32)
    w_k = np.asarray(w_k, dtype=np.float32)
    w_v = np.asarray(w_v, dtype=np.float32)
    w_o = np.asarray(w_o, dtype=np.float32)
    b_o = np.asarray(b_o, dtype=np.float32)
    Bc, Sc, Dc = x.shape
    in_maps = make_in_maps(x, w_q, w_k, w_v, w_o)
    res = run_on_hw(in_maps, Sc)
    out = np.empty((Bc, Sc, Dc), dtype=np.float32)
    for b in range(Bc):
        outT = res.results[2 * b]["outT"] + res.results[2 * b + 1]["outT"]
        out[b] = outT.T + b_o
    return out
